# revision 14
# baseline (speedup 1.0000x reference)
"""BlockSparseMLP (MoE top-2 routing, 8 experts) — Trainium2 Bass kernel.

v2: weights and activations pre-cast to bf16 on the HOST, so HBM traffic
is halved (69 MB/core of weights instead of 138 MB).  Everything else as
the baseline: expert-per-core, host router/dispatch/combine, device runs
the gated MLP with fp32 PSUM accumulation.
"""

import os

import numpy as np
import ml_dtypes

T, D, F, E, TOPK = 2048, 2048, 5632, 8, 2
P = 128
KD = D // P     # 16 k-subtiles over D
KF = F // P     # 44 k-subtiles over F
FG = 4          # f-tiles per phase-1 weight DMA block (512 F columns)
NFG = KF // FG  # 11 phase-1 blocks
DG = 2          # d-tiles per phase-2 psum group (256 D columns)
NDG = KD // DG  # 8 phase-2 d-groups
KO2 = 4         # f-subtiles per phase-2 weight DMA block
NFB = KF // KO2  # 11 phase-2 blocks per d-group

_COMPILED = {}   # CAP -> (nc, chunk list)
LAST_RESULT = None  # BassKernelResults of the most recent run (for test.py)

BF16 = ml_dtypes.bfloat16


def _to_bf16(a):
    """fp32 ndarray -> bf16 (RNE), vectorized."""
    u = np.ascontiguousarray(a, dtype=np.float32).view(np.uint32)
    r = ((u + np.uint32(0x7FFF) + ((u >> np.uint32(16)) & np.uint32(1)))
         >> np.uint32(16)).astype(np.uint16)
    return r.view(BF16)


def _token_chunks(cap):
    """Split cap into free-dim chunks, each in [256, 512]."""
    assert cap >= 512 and cap % 2 == 0
    n512, rem = divmod(cap, 512)
    if rem == 0:
        return [512] * n512
    if rem >= 256:
        return [512] * n512 + [rem]
    return [512] * (n512 - 1) + [256 + rem, 256]


def _build(cap):
    """Build + compile the SPMD Tile program for token capacity `cap`."""
    import concourse.bass as bass  # noqa: F401
    import concourse.mybir as mybir
    import concourse.tile as tile
    from concourse import bacc

    f32 = mybir.dt.float32
    bf16 = mybir.dt.bfloat16
    mult = mybir.AluOpType.mult

    chunks = _token_chunks(cap)
    starts = [sum(chunks[:i]) for i in range(len(chunks))]

    nc = bacc.Bacc("TRN2", target_bir_lowering=False, debug=False,
                   enable_asserts=False, num_devices=E)

    xt_d = nc.dram_tensor("xt", [P, KD, cap], bf16, kind="ExternalInput").ap()
    wg_d = nc.dram_tensor("wg", [NFG, P, KD, P * FG], bf16,
                          kind="ExternalInput").ap()
    wu_d = nc.dram_tensor("wu", [NFG, P, KD, P * FG], bf16,
                          kind="ExternalInput").ap()
    wd_d = nc.dram_tensor("wd", [NDG, NFB, P, KO2, P * DG], bf16,
                          kind="ExternalInput").ap()
    wr_d = nc.dram_tensor("wrep", [P, cap], f32, kind="ExternalInput").ap()
    out_d = nc.dram_tensor("out_t", [D, cap], f32, kind="ExternalOutput").ap()
    scr_d = nc.dram_tensor("scr", [P, 512], f32).ap()   # warm-up sink
    scr2_d = nc.dram_tensor("scr2", [P, 512], f32).ap()  # warm-up source (garbage ok)

    with tile.TileContext(nc) as tc:
        with (
            tc.tile_pool(name="resident", bufs=1) as rpool,
            tc.tile_pool(name="w1", bufs=3) as w1pool,
            tc.tile_pool(name="wd2", bufs=8) as wd2pool,
            tc.tile_pool(name="outp", bufs=4) as outpool,
            tc.tile_pool(name="psum", bufs=2, space="PSUM") as ppool,
        ):
            xt = rpool.tile([P, KD, cap], bf16)
            wrep = rpool.tile([P, cap], f32)
            nc.sync.dma_start(wrep[:], wr_d)
            at = rpool.tile([P, KF, cap], bf16)

            # Warm-up: run throwaway matmuls while the first DMAs are in
            # flight so the PE HAM clock-gate opens (1.2 -> 2.4 GHz)
            # before real work arrives.
            warm = rpool.tile([P, 512], bf16)
            # garbage-initialize via a tiny DMA (cast f32->bf16): the PE can
            # start warm-up ~2.5us in, vs ~8.4us gated on a DVE memset
            nc.gpsimd.dma_start(warm[:], scr2_d)
            wps = ppool.tile([P, 512], f32, tag="ps0c0", name="warm_ps")
            NWARM = 14
            for i in range(NWARM):
                nc.tensor.matmul(wps[:], warm[:, :P], warm[:],
                                 start=(i == 0), stop=(i == NWARM - 1))
            wout = rpool.tile([P, 512], f32)
            nc.vector.tensor_copy(out=wout[:], in_=wps[:])
            nc.sync.dma_start(scr_d[:], wout[:])

            # Queue order on the single SWDGE ring decides arrival order:
            # first weight sub-block + first token slices (so PE can start
            # early), then the token bulk, then the stream.
            nc.gpsimd.dma_start(xt[:], xt_d)

            w1tiles = []
            for fg in range(NFG):
                wgb = w1pool.tile([P, KD, P * FG], bf16, tag="wgb",
                                  name=f"wgb_{fg}")
                wub = w1pool.tile([P, KD, P * FG], bf16, tag="wub",
                                  name=f"wub_{fg}")
                w1tiles.append((wgb, wub))
                if fg == 0:
                    nc.gpsimd.dma_start(wgb[:], wg_d[0])
                    nc.gpsimd.dma_start(wub[:], wu_d[0])
                else:
                    kh = KD // 2
                    nc.gpsimd.dma_start(wgb[:, :kh, :], wg_d[fg][:, :kh, :])
                    nc.gpsimd.dma_start(wgb[:, kh:, :], wg_d[fg][:, kh:, :])
                    nc.gpsimd.dma_start(wub[:, :kh, :], wu_d[fg][:, :kh, :])
                    nc.gpsimd.dma_start(wub[:, kh:, :], wu_d[fg][:, kh:, :])

                # ---- phase 1: gT/uT = W.T @ xT, aT = silu(gT)*uT ----
                for fs in range(FG):
                    ft = fg * FG + fs
                    for ci, (c0, cn) in enumerate(zip(starts, chunks)):
                        pg = ppool.tile([P, cn], f32, tag=f"ps0c{ci}")
                        pu = ppool.tile([P, cn], f32, tag=f"ps1c{ci}")
                        for ko in range(KD):
                            nc.tensor.matmul(
                                pg[:], wgb[:, ko, fs * P:(fs + 1) * P],
                                xt[:, ko, c0:c0 + cn],
                                start=(ko == 0), stop=(ko == KD - 1))
                        for ko in range(KD):
                            nc.tensor.matmul(
                                pu[:], wub[:, ko, fs * P:(fs + 1) * P],
                                xt[:, ko, c0:c0 + cn],
                                start=(ko == 0), stop=(ko == KD - 1))
                        a_sl = at[:, ft, c0:c0 + cn]
                        nc.scalar.activation(
                            a_sl, pg[:], mybir.ActivationFunctionType.Silu)
                        nc.vector.tensor_tensor(a_sl, a_sl, pu[:], mult)

            # ---- phase 2: dT = Wd.T @ aT, out = dT * w ----
            for dg in range(NDG):
                pds = [[ppool.tile([P, cn], f32, tag=f"ps{ds}c{ci}",
                                   name=f"pd_{dg}_{ds}_{ci}")
                        for ci, cn in enumerate(chunks)]
                       for ds in range(DG)]
                for fb in range(NFB):
                    wdb = wd2pool.tile([P, KO2, P * DG], bf16, tag="wdb")
                    nc.gpsimd.dma_start(wdb[:], wd_d[dg, fb])
                    for ko in range(KO2):
                        fk = fb * KO2 + ko
                        for ds in range(DG):
                            for ci, (c0, cn) in enumerate(zip(starts, chunks)):
                                nc.tensor.matmul(
                                    pds[ds][ci][:],
                                    wdb[:, ko, ds * P:(ds + 1) * P],
                                    at[:, fk, c0:c0 + cn],
                                    start=(fk == 0), stop=(fk == KF - 1))
                for ds in range(DG):
                    ot = outpool.tile([P, cap], f32, tag="ot")
                    for ci, (c0, cn) in enumerate(zip(starts, chunks)):
                        nc.vector.tensor_tensor(
                            ot[:, c0:c0 + cn], pds[ds][ci][:],
                            wrep[:, c0:c0 + cn], mult)
                    dt_idx = dg * DG + ds
                    nc.sync.dma_start(
                        out_d[dt_idx * P:(dt_idx + 1) * P, :], ot[:])

    nc.compile()
    return nc, chunks


def _swizzle_w1(w):
    """bf16 [D, F] -> [NFG, P, KD, P*FG] block-major, partition-contiguous."""
    return np.ascontiguousarray(
        w.reshape(KD, P, NFG, P * FG).transpose(2, 1, 0, 3))


def _swizzle_wd(w):
    """bf16 [F, D] -> [NDG, NFB, P, KO2, P*DG] block-major."""
    return np.ascontiguousarray(
        w.reshape(NFB, KO2, P, NDG, P * DG).transpose(3, 0, 2, 1, 4))


def kernel(x, gate_tensor, Wg, Wu, Wd):
    global LAST_RESULT
    from concourse.bass_interp import get_hw_module
    from concourse.bass_utils import run_bass_kernel_spmd

    x = np.ascontiguousarray(np.asarray(x, dtype=np.float32))
    gate_tensor = np.asarray(gate_tensor, dtype=np.float32)

    # ---- router (replicated; tiny: T*D*E flops) ----
    logits = x @ gate_tensor                      # [T, E] fp32
    m = logits.max(axis=-1, keepdims=True)
    p = np.exp(logits - m, dtype=np.float32)
    p /= p.sum(axis=-1, keepdims=True)
    topi = np.argsort(-p, axis=-1, kind="stable")[:, :TOPK]      # [T, K]
    topw = np.take_along_axis(p, topi, axis=-1)
    topw = topw / (topw.sum(axis=-1, keepdims=True) + 1e-20)

    idx = []          # tokens routed to each expert
    wts = []          # their combine weights
    for e in range(E):
        sel = (topi == e)                         # [T, K]; <=1 True per row
        idx.append(np.nonzero(sel.any(axis=-1))[0])
        wts.append(topw[sel].astype(np.float32))  # row-major == token order
    max_n = max(len(t) for t in idx)
    cap = max(512, ((max_n + 1) // 2) * 2)

    if cap not in _COMPILED:
        _COMPILED[cap] = _build(cap)
    nc, _chunks = _COMPILED[cap]

    xb = _to_bf16(x)                              # [T, D] bf16

    # ---- dispatch: per-core inputs (pre-swizzled to SBUF block layout) ----
    in_maps = []
    for e in range(E):
        n = len(idx[e])
        xg = xb[idx[e]]                           # [n, D] bf16
        xt = np.zeros((P, KD, cap), dtype=BF16)
        xt[:, :, :n] = xg.T.reshape(KD, P, n).transpose(1, 0, 2)
        wr = np.zeros((P, cap), dtype=np.float32)
        wr[:, :n] = wts[e][None, :]
        in_maps.append({"xt": xt,
                        "wg": _swizzle_w1(_to_bf16(Wg[e])),
                        "wu": _swizzle_w1(_to_bf16(Wu[e])),
                        "wd": _swizzle_wd(_to_bf16(Wd[e])),
                        "wrep": wr})

    trace = bool(int(os.environ.get("KERNEL_TRACE", "0")))
    old_m = nc.m
    nc.m = get_hw_module(nc.m)
    try:
        try:
            res = run_bass_kernel_spmd(nc, in_maps, core_ids=list(range(E)),
                                       trace=trace)
        except (ImportError, ModuleNotFoundError):
            os.environ["BASS_NEVER_TRACE"] = "1"
            res = run_bass_kernel_spmd(nc, in_maps, core_ids=list(range(E)),
                                       trace=False)
    finally:
        nc.m = old_m
    LAST_RESULT = res

    # ---- combine: scatter-add the per-expert partials ----
    out = np.zeros((T, D), dtype=np.float32)
    for e in range(E):
        n = len(idx[e])
        out[idx[e]] += res.results[e]["out_t"][:, :n].T
    return out


# revision 15
# speedup vs baseline: 1.0346x; 1.0346x over previous
"""BlockSparseMLP (MoE top-2 routing, 8 experts) — Trainium2 Bass kernel.

v2: weights and activations pre-cast to bf16 on the HOST, so HBM traffic
is halved (69 MB/core of weights instead of 138 MB).  Everything else as
the baseline: expert-per-core, host router/dispatch/combine, device runs
the gated MLP with fp32 PSUM accumulation.
"""

import os

import numpy as np
import ml_dtypes

T, D, F, E, TOPK = 2048, 2048, 5632, 8, 2
P = 128
KD = D // P     # 16 k-subtiles over D
KF = F // P     # 44 k-subtiles over F
FG = 4          # f-tiles per phase-1 weight DMA block (512 F columns)
NFG = KF // FG  # 11 phase-1 blocks
DG = 2          # d-tiles per phase-2 psum group (256 D columns)
NDG = KD // DG  # 8 phase-2 d-groups
KO2 = 4         # f-subtiles per phase-2 weight DMA block
NFB = KF // KO2  # 11 phase-2 blocks per d-group

_COMPILED = {}   # CAP -> (nc, chunk list)
LAST_RESULT = None  # BassKernelResults of the most recent run (for test.py)

BF16 = ml_dtypes.bfloat16


def _to_bf16(a):
    """fp32 ndarray -> bf16 (RNE), vectorized."""
    u = np.ascontiguousarray(a, dtype=np.float32).view(np.uint32)
    r = ((u + np.uint32(0x7FFF) + ((u >> np.uint32(16)) & np.uint32(1)))
         >> np.uint32(16)).astype(np.uint16)
    return r.view(BF16)


def _token_chunks(cap):
    """Split cap into free-dim chunks, each in [256, 512]."""
    assert cap >= 512 and cap % 2 == 0
    n512, rem = divmod(cap, 512)
    if rem == 0:
        return [512] * n512
    if rem >= 256:
        return [512] * n512 + [rem]
    return [512] * (n512 - 1) + [256 + rem, 256]


def _build(cap):
    """Build + compile the SPMD Tile program for token capacity `cap`."""
    import concourse.bass as bass  # noqa: F401
    import concourse.mybir as mybir
    import concourse.tile as tile
    from concourse import bacc

    f32 = mybir.dt.float32
    bf16 = mybir.dt.bfloat16
    mult = mybir.AluOpType.mult

    chunks = _token_chunks(cap)
    starts = [sum(chunks[:i]) for i in range(len(chunks))]

    nc = bacc.Bacc("TRN2", target_bir_lowering=False, debug=False,
                   enable_asserts=False, num_devices=E)

    xt_d = nc.dram_tensor("xt", [P, KD, cap], bf16, kind="ExternalInput").ap()
    wg_d = nc.dram_tensor("wg", [NFG, P, KD, P * FG], bf16,
                          kind="ExternalInput").ap()
    wu_d = nc.dram_tensor("wu", [NFG, P, KD, P * FG], bf16,
                          kind="ExternalInput").ap()
    wd_d = nc.dram_tensor("wd", [NDG, NFB, P, KO2, P * DG], bf16,
                          kind="ExternalInput").ap()
    wr_d = nc.dram_tensor("wrep", [P, cap], f32, kind="ExternalInput").ap()
    out_d = nc.dram_tensor("out_t", [D, cap], f32, kind="ExternalOutput").ap()
    scr_d = nc.dram_tensor("scr", [P, 512], f32).ap()   # warm-up sink

    with tile.TileContext(nc) as tc:
        with (
            tc.tile_pool(name="resident", bufs=1) as rpool,
            tc.tile_pool(name="w1", bufs=3) as w1pool,
            tc.tile_pool(name="wd2", bufs=8) as wd2pool,
            tc.tile_pool(name="outp", bufs=4) as outpool,
            tc.tile_pool(name="psum", bufs=2, space="PSUM") as ppool,
        ):
            xt = rpool.tile([P, KD, cap], bf16)
            wrep = rpool.tile([P, cap], f32)
            nc.sync.dma_start(wrep[:], wr_d)
            at = rpool.tile([P, KF, cap], bf16)

            # Warm-up: run throwaway matmuls while the first DMAs are in
            # flight so the PE HAM clock-gate opens (1.2 -> 2.4 GHz)
            # before real work arrives.
            warm = rpool.tile([P, 512], bf16)
            nc.vector.memset(warm[:], 0.0)
            wps = ppool.tile([P, 512], f32, tag="ps0c0", name="warm_ps")
            for i in range(20):
                nc.tensor.matmul(wps[:], warm[:, :P], warm[:],
                                 start=(i == 0), stop=(i == 19))
            wout = rpool.tile([P, 512], f32)
            nc.vector.tensor_copy(out=wout[:], in_=wps[:])
            nc.sync.dma_start(scr_d[:], wout[:])

            # Queue order on the single SWDGE ring decides arrival order:
            # first weight sub-block + first token slices (so PE can start
            # early), then the token bulk, then the stream.
            nc.gpsimd.dma_start(xt[:, :2, :], xt_d[:, :2, :])

            w1tiles = []
            for fg in range(NFG):
                wgb = w1pool.tile([P, KD, P * FG], bf16, tag="wgb",
                                  name=f"wgb_{fg}")
                wub = w1pool.tile([P, KD, P * FG], bf16, tag="wub",
                                  name=f"wub_{fg}")
                w1tiles.append((wgb, wub))
                if fg == 0:
                    # fine-grained first block + token bulk spread over
                    # several DMAs so multiple SWDGE lanes pull in parallel
                    for s in range(FG):
                        sl = slice(s * P, (s + 1) * P)
                        nc.gpsimd.dma_start(wgb[:, :, sl], wg_d[0][:, :, sl])
                        nc.gpsimd.dma_start(wub[:, :, sl], wu_d[0][:, :, sl])
                        if s == 0:
                            for k0 in range(2, KD, 2):
                                nc.gpsimd.dma_start(
                                    xt[:, k0:k0 + 2, :], xt_d[:, k0:k0 + 2, :])
                else:
                    kh = KD // 2
                    nc.gpsimd.dma_start(wgb[:, :kh, :], wg_d[fg][:, :kh, :])
                    nc.gpsimd.dma_start(wgb[:, kh:, :], wg_d[fg][:, kh:, :])
                    nc.gpsimd.dma_start(wub[:, :kh, :], wu_d[fg][:, :kh, :])
                    nc.gpsimd.dma_start(wub[:, kh:, :], wu_d[fg][:, kh:, :])

                # ---- phase 1: gT/uT = W.T @ xT, aT = silu(gT)*uT ----
                for fs in range(FG):
                    ft = fg * FG + fs
                    for ci, (c0, cn) in enumerate(zip(starts, chunks)):
                        pg = ppool.tile([P, cn], f32, tag=f"ps0c{ci}")
                        pu = ppool.tile([P, cn], f32, tag=f"ps1c{ci}")
                        for ko in range(KD):
                            nc.tensor.matmul(
                                pg[:], wgb[:, ko, fs * P:(fs + 1) * P],
                                xt[:, ko, c0:c0 + cn],
                                start=(ko == 0), stop=(ko == KD - 1))
                        for ko in range(KD):
                            nc.tensor.matmul(
                                pu[:], wub[:, ko, fs * P:(fs + 1) * P],
                                xt[:, ko, c0:c0 + cn],
                                start=(ko == 0), stop=(ko == KD - 1))
                        a_sl = at[:, ft, c0:c0 + cn]
                        nc.scalar.activation(
                            a_sl, pg[:], mybir.ActivationFunctionType.Silu)
                        nc.vector.tensor_tensor(a_sl, a_sl, pu[:], mult)

            # ---- phase 2: dT = Wd.T @ aT, out = dT * w ----
            for dg in range(NDG):
                pds = [[ppool.tile([P, cn], f32, tag=f"ps{ds}c{ci}",
                                   name=f"pd_{dg}_{ds}_{ci}")
                        for ci, cn in enumerate(chunks)]
                       for ds in range(DG)]
                for fb in range(NFB):
                    wdb = wd2pool.tile([P, KO2, P * DG], bf16, tag="wdb")
                    nc.gpsimd.dma_start(wdb[:], wd_d[dg, fb])
                    for ko in range(KO2):
                        fk = fb * KO2 + ko
                        for ds in range(DG):
                            for ci, (c0, cn) in enumerate(zip(starts, chunks)):
                                nc.tensor.matmul(
                                    pds[ds][ci][:],
                                    wdb[:, ko, ds * P:(ds + 1) * P],
                                    at[:, fk, c0:c0 + cn],
                                    start=(fk == 0), stop=(fk == KF - 1))
                for ds in range(DG):
                    ot = outpool.tile([P, cap], f32, tag="ot")
                    for ci, (c0, cn) in enumerate(zip(starts, chunks)):
                        nc.vector.tensor_tensor(
                            ot[:, c0:c0 + cn], pds[ds][ci][:],
                            wrep[:, c0:c0 + cn], mult)
                    dt_idx = dg * DG + ds
                    nc.sync.dma_start(
                        out_d[dt_idx * P:(dt_idx + 1) * P, :], ot[:])

    nc.compile()
    return nc, chunks


def _swizzle_w1(w):
    """bf16 [D, F] -> [NFG, P, KD, P*FG] block-major, partition-contiguous."""
    return np.ascontiguousarray(
        w.reshape(KD, P, NFG, P * FG).transpose(2, 1, 0, 3))


def _swizzle_wd(w):
    """bf16 [F, D] -> [NDG, NFB, P, KO2, P*DG] block-major."""
    return np.ascontiguousarray(
        w.reshape(NFB, KO2, P, NDG, P * DG).transpose(3, 0, 2, 1, 4))


def kernel(x, gate_tensor, Wg, Wu, Wd):
    global LAST_RESULT
    from concourse.bass_interp import get_hw_module
    from concourse.bass_utils import run_bass_kernel_spmd

    x = np.ascontiguousarray(np.asarray(x, dtype=np.float32))
    gate_tensor = np.asarray(gate_tensor, dtype=np.float32)

    # ---- router (replicated; tiny: T*D*E flops) ----
    logits = x @ gate_tensor                      # [T, E] fp32
    m = logits.max(axis=-1, keepdims=True)
    p = np.exp(logits - m, dtype=np.float32)
    p /= p.sum(axis=-1, keepdims=True)
    topi = np.argsort(-p, axis=-1, kind="stable")[:, :TOPK]      # [T, K]
    topw = np.take_along_axis(p, topi, axis=-1)
    topw = topw / (topw.sum(axis=-1, keepdims=True) + 1e-20)

    idx = []          # tokens routed to each expert
    wts = []          # their combine weights
    for e in range(E):
        sel = (topi == e)                         # [T, K]; <=1 True per row
        idx.append(np.nonzero(sel.any(axis=-1))[0])
        wts.append(topw[sel].astype(np.float32))  # row-major == token order
    max_n = max(len(t) for t in idx)
    cap = max(512, ((max_n + 1) // 2) * 2)

    if cap not in _COMPILED:
        _COMPILED[cap] = _build(cap)
    nc, _chunks = _COMPILED[cap]

    xb = _to_bf16(x)                              # [T, D] bf16

    # ---- dispatch: per-core inputs (pre-swizzled to SBUF block layout) ----
    in_maps = []
    for e in range(E):
        n = len(idx[e])
        xg = xb[idx[e]]                           # [n, D] bf16
        xt = np.zeros((P, KD, cap), dtype=BF16)
        xt[:, :, :n] = xg.T.reshape(KD, P, n).transpose(1, 0, 2)
        wr = np.zeros((P, cap), dtype=np.float32)
        wr[:, :n] = wts[e][None, :]
        in_maps.append({"xt": xt,
                        "wg": _swizzle_w1(_to_bf16(Wg[e])),
                        "wu": _swizzle_w1(_to_bf16(Wu[e])),
                        "wd": _swizzle_wd(_to_bf16(Wd[e])),
                        "wrep": wr})

    trace = bool(int(os.environ.get("KERNEL_TRACE", "0")))
    old_m = nc.m
    nc.m = get_hw_module(nc.m)
    try:
        try:
            res = run_bass_kernel_spmd(nc, in_maps, core_ids=list(range(E)),
                                       trace=trace)
        except (ImportError, ModuleNotFoundError):
            os.environ["BASS_NEVER_TRACE"] = "1"
            res = run_bass_kernel_spmd(nc, in_maps, core_ids=list(range(E)),
                                       trace=False)
    finally:
        nc.m = old_m
    LAST_RESULT = res

    # ---- combine: scatter-add the per-expert partials ----
    out = np.zeros((T, D), dtype=np.float32)
    for e in range(E):
        n = len(idx[e])
        out[idx[e]] += res.results[e]["out_t"][:, :n].T
    return out
